# revision 24
# baseline (speedup 1.0000x reference)
"""Trainium2 Bass kernel for nn_DeltaRuleModel (scatter_memory).

Model: token embed -> per-token MLP+LayerNorm encoder -> sequential
delta-rule memory scan over L-1 steps -> readout of the final memory
against the last position's hidden -> 2 small dense layers.

Key algebraic facts exploited:
  1. The encoder output hidden[b, l] depends only on the token id
     seq[b, l]  =>  the whole encoder collapses to a 64x32 table (TBL),
     computed on the host from the small weights (pure weight
     preprocessing; all per-token work stays on device).
  2. The scan M <- M (I - a k k^T) + k k^T with the final readout
     y = M_T q is linear in M, so y equals a backward *vector*
     recurrence (no 32x32 matrix state):
         u <- q;  for s = T..1:  d = k_s.u ; y += d k_s ; u -= a_s d k_s
     This is 2 fused DVE ops per step on [128, 32] tiles (batch on
     partitions) instead of a 32x32 matrix update.

Per-core dataflow (128 batch lanes on partitions):
  - GPSIMD builds one-hot selectors from replicated token ids
    (is_equal against a per-partition iota), f32.
  - PE turns each step's one-hot [64v x 128b] into the k-slab
    [128b x (ktilde|k)] via a matmul against the 64x64 table ->
    an on-chip "gather" at matmul speed, no DMA descriptors.
  - ACT drains PSUM k-slabs to SBUF once per chunk.
  - DVE runs the sequential scan: per step one fused multiply+reduce
    (d = k.u) and one fused multiply+add (u += d*ktilde_neg); y is
    accumulated once per chunk from the stored d's.
"""

import numpy as np

B, L, H, V = 1024, 2048, 32, 64
N_CORES = 8
BL = B // N_CORES          # 128 batch lanes per core
T = L - 1                  # 2047 scan steps (keys = positions 0..L-2)
W = 8                      # steps per chunk (one PSUM bank = 8*64 f32)
LN_EPS = 1e-5
DELTA_EPS = 1e-6

_BUILT = {}


def _build_module(t_steps=T, w=W):
    """Build the Bass module (once per process)."""
    import concourse.bass as bass  # noqa: F401
    import concourse.mybir as mybir
    import concourse.tile as tile
    from concourse import bacc
    from concourse.masks import make_identity

    f32 = mybir.dt.float32
    bf16 = mybir.dt.bfloat16
    OP = mybir.AluOpType

    nc = bacc.Bacc("TRN2", target_bir_lowering=False, debug=False,
                   num_devices=N_CORES)

    n_chunks = (t_steps + w - 1) // w
    ncols = n_chunks * w * BL  # one column per (step, batch), padded

    tok = nc.dram_tensor("tok", [V, ncols], bf16, kind="ExternalInput")
    tbl = nc.dram_tensor("tbl", [V, 2 * H], f32, kind="ExternalInput")
    iot = nc.dram_tensor("iot", [V, 1], f32, kind="ExternalInput")
    qin = nc.dram_tensor("qin", [BL, H], f32, kind="ExternalInput")
    rw = nc.dram_tensor("rw", [H, H], f32, kind="ExternalInput")
    rb = nc.dram_tensor("rb", [H, 1], f32, kind="ExternalInput")
    ow = nc.dram_tensor("ow", [H, V], f32, kind="ExternalInput")
    ob = nc.dram_tensor("ob", [V, 1], f32, kind="ExternalInput")
    outT = nc.dram_tensor("outT", [V, BL], f32, kind="ExternalOutput")

    cw = w * BL  # token columns per chunk

    with tile.TileContext(nc) as tc:
        with (
            tc.tile_pool(name="persist", bufs=1) as persist,
            tc.tile_pool(name="tokp", bufs=4) as tokp,
            tc.tile_pool(name="ohp", bufs=4) as ohp,
            tc.tile_pool(name="kp", bufs=4) as kp,
            tc.tile_pool(name="dpool", bufs=2) as dpool,
            tc.tile_pool(name="spool", bufs=2) as spool,
            tc.tile_pool(name="ypool", bufs=2) as ypool,
            tc.tile_pool(name="psum", bufs=4, space="PSUM") as psum,
            tc.tile_pool(name="psum_r", bufs=1, space="PSUM") as psum_r,
        ):
            u = persist.tile([BL, H], f32)
            nc.sync.dma_start(u[:], qin.ap())
            y = persist.tile([BL, H], f32)
            nc.vector.memset(y[:], 0.0)
            tbl_sb = persist.tile([V, 2 * H], f32)
            nc.sync.dma_start(tbl_sb[:], tbl.ap())
            iota_sb = persist.tile([V, 1], f32)
            nc.sync.dma_start(iota_sb[:], iot.ap())

            rw_sb = persist.tile([H, H], f32)
            nc.sync.dma_start(rw_sb[:], rw.ap())
            rb_sb = persist.tile([H, 1], f32)
            nc.sync.dma_start(rb_sb[:], rb.ap())
            ow_sb = persist.tile([H, V], f32)
            nc.sync.dma_start(ow_sb[:], ow.ap())
            ob_sb = persist.tile([V, 1], f32)
            nc.sync.dma_start(ob_sb[:], ob.ap())
            ident = persist.tile([BL, BL], f32)
            make_identity(nc, ident[:])

            for c in range(n_chunks):
                wc = min(w, t_steps - c * w)
                # token ids for this chunk, replicated across 64 partitions
                tk = tokp.tile([V, cw], bf16, tag="tk")
                nc.sync.dma_start(tk[:], tok.ap()[:, c * cw:(c + 1) * cw])
                # one-hot selectors (f32 0/1), built on GPSIMD
                oh = ohp.tile([V, cw], f32, tag="oh")
                nc.gpsimd.tensor_scalar(
                    out=oh[:], in0=tk[:], scalar1=iota_sb[:, 0:1],
                    scalar2=None, op0=OP.is_equal)
                # PE: one matmul per step -> k-slab [128b, ktilde|k] in PSUM
                kps = psum.tile([BL, w, 2 * H], f32, tag="kps")
                for j in range(wc):
                    nc.tensor.matmul(
                        out=kps[:, j, :],
                        lhsT=oh[:, j * BL:(j + 1) * BL],
                        rhs=tbl_sb[:],
                        start=True, stop=True)
                # drain chunk to SBUF (scalar engine)
                kt = kp.tile([BL, w, 2 * H], f32, tag="kt")
                nc.scalar.copy(out=kt[:, :wc, :], in_=kps[:, :wc, :])

                db = dpool.tile([BL, w], f32, tag="db")
                for j in range(wc):
                    sc = spool.tile([BL, H], f32, tag="sc")
                    # d_j = sum_h k*u   (out tile is scratch)
                    nc.vector.scalar_tensor_tensor(
                        out=sc[:], in0=kt[:, j, H:2 * H], scalar=1.0,
                        in1=u[:], op0=OP.mult, op1=OP.mult,
                        accum_out=db[:, j:j + 1],
                    )
                    # u += d_j * ktilde_neg_j
                    nc.vector.scalar_tensor_tensor(
                        out=u[:], in0=kt[:, j, 0:H], scalar=db[:, j:j + 1],
                        in1=u[:], op0=OP.mult, op1=OP.add,
                    )
                # y accumulation per chunk: y += sum_j d_j * k_j
                yt = ypool.tile([BL, H, w], f32, tag="yt")
                d_b = db[:, 0:wc].rearrange(
                    "p (s o) -> p o s", o=1).to_broadcast([BL, H, wc])
                k_b = kt[:, 0:wc, H:2 * H].rearrange("p s h -> p h s")
                nc.vector.tensor_tensor(
                    out=yt[:, :, :wc], in0=d_b, in1=k_b, op=OP.mult)
                yp = spool.tile([BL, H], f32, tag="yp")
                nc.vector.tensor_reduce(
                    out=yp[:], in_=yt[:, :, :wc],
                    axis=mybir.AxisListType.X, op=OP.add)
                nc.vector.tensor_tensor(
                    out=y[:], in0=y[:], in1=yp[:], op=OP.add)

            # ---- readout: out = (y @ rw + rb) @ ow + ob, emitted transposed
            yT_ps = psum_r.tile([H, BL], f32, tag="yT")
            nc.tensor.transpose(out=yT_ps[:], in_=y[:], identity=ident[:])
            yT = spool.tile([H, BL], f32, tag="yT_sb")
            nc.scalar.copy(out=yT[:], in_=yT_ps[:])

            r1_ps = psum_r.tile([H, BL], f32, tag="r1")
            nc.tensor.matmul(out=r1_ps[:], lhsT=rw_sb[:], rhs=yT[:],
                             start=True, stop=True)
            r1 = spool.tile([H, BL], f32, tag="r1_sb")
            nc.scalar.add(out=r1[:], in_=r1_ps[:], add=rb_sb[:])

            o_ps = psum_r.tile([V, BL], f32, tag="o")
            nc.tensor.matmul(out=o_ps[:], lhsT=ow_sb[:], rhs=r1[:],
                             start=True, stop=True)
            o_sb = spool.tile([V, BL], f32, tag="o_sb")
            nc.scalar.add(out=o_sb[:], in_=o_ps[:], add=ob_sb[:])
            nc.sync.dma_start(outT.ap(), o_sb[:])

    nc.compile()
    return nc


def _host_tables(embed, w1, b1, w2, b2, ln_g, ln_b):
    """64x32 encoder LUT + the [ -a*k | k ] table, all f32."""
    f = np.float32
    h = embed.astype(f)                      # [64, 32] (ids 0..63)
    ff = np.maximum(h @ w1.astype(f) + b1.astype(f), f(0)) @ w2.astype(f) \
        + b2.astype(f)
    x = h + ff
    mu = x.mean(-1, keepdims=True, dtype=f)
    var = ((x - mu) ** 2).mean(-1, keepdims=True, dtype=f)
    lut = ((x - mu) / np.sqrt(var + f(LN_EPS)) * ln_g.astype(f)
           + ln_b.astype(f)).astype(f)       # [64, 32]
    alpha = f(1.0) / ((lut * lut).sum(-1) + f(DELTA_EPS))   # [64]
    tbl = np.concatenate([-alpha[:, None] * lut, lut], axis=1).astype(f)
    return lut, tbl


def kernel(seq, embed, w1, b1, w2, b2, ln_g, ln_b, read_w, read_b,
           out_w, out_b):
    import ml_dtypes
    from concourse.bass_utils import run_bass_kernel_spmd

    seq = np.asarray(seq)
    lut, tbl = _host_tables(np.asarray(embed), np.asarray(w1), np.asarray(b1),
                            np.asarray(w2), np.asarray(b2),
                            np.asarray(ln_g), np.asarray(ln_b))

    # reversed key order: column g holds the token at position L-2-g
    keys_rev = seq[:, L - 2::-1].astype(np.int32)        # [B, T]
    q_all = lut[seq[:, L - 1]]                           # [B, H] f32

    n_chunks = (T + W - 1) // W
    L2 = n_chunks * W

    rw_np = np.asarray(read_w, np.float32)
    rb_np = np.asarray(read_b, np.float32).reshape(H, 1)
    ow_np = np.asarray(out_w, np.float32)
    ob_np = np.asarray(out_b, np.float32).reshape(V, 1)
    iota = np.arange(V, dtype=np.float32).reshape(V, 1)

    if "nc" not in _BUILT:
        _BUILT["nc"] = _build_module()
    nc = _BUILT["nc"]

    in_maps = []
    for c in range(N_CORES):
        sl = slice(c * BL, (c + 1) * BL)
        kr = np.full((BL, L2), -1, np.int32)
        kr[:, :T] = keys_rev[sl]
        # column order: g-major, batch-minor; replicated over 64 partitions
        cols = kr.T.ravel().astype(np.float32).astype(ml_dtypes.bfloat16)
        tok = np.broadcast_to(cols[None, :], (V, L2 * BL))
        in_maps.append({
            "tok": np.ascontiguousarray(tok),
            "tbl": tbl,
            "iot": iota,
            "qin": np.ascontiguousarray(q_all[sl]),
            "rw": rw_np, "rb": rb_np, "ow": ow_np, "ob": ob_np,
        })

    import os
    trace = os.environ.get("KERNEL_TRACE", "0") == "1"
    res = run_bass_kernel_spmd(nc, in_maps, core_ids=list(range(N_CORES)),
                               trace=trace)
    _BUILT["last_result"] = res
    out = np.empty((B, V), np.float32)
    for c in range(N_CORES):
        out[c * BL:(c + 1) * BL] = res.results[c]["outT"].T
    return out


# revision 27
# speedup vs baseline: 3.0223x; 3.0223x over previous
"""Trainium2 Bass kernel for nn_DeltaRuleModel (scatter_memory).

Model: token embed -> per-token MLP+LayerNorm encoder -> sequential
delta-rule memory scan over L-1 steps -> readout of the final memory
against the last position's hidden -> 2 small dense layers.

Key algebraic facts exploited:
  1. The encoder output hidden[b, l] depends only on the token id
     seq[b, l]  =>  the whole encoder collapses to a 64x32 table (TBL),
     computed on the host from the small weights (pure weight
     preprocessing; all per-token work stays on device).
  2. The scan M <- M (I - a k k^T) + k k^T with the final readout
     y = M_T q is linear in M, so y equals a backward *vector*
     recurrence (no 32x32 matrix state):
         u <- q;  for s = T..1:  d = k_s.u ; y += d k_s ; u -= a_s d k_s
     This is 2 fused DVE ops per step on [128, 32] tiles (batch on
     partitions) instead of a 32x32 matrix update.

Per-core dataflow (128 batch lanes on partitions):
  - GPSIMD builds one-hot selectors from replicated token ids
    (is_equal against a per-partition iota), f32.
  - PE turns each step's one-hot [64v x 128b] into the k-slab
    [128b x (ktilde|k)] via a matmul against the 64x64 table ->
    an on-chip "gather" at matmul speed, no DMA descriptors.
  - ACT drains PSUM k-slabs to SBUF once per chunk.
  - DVE runs the sequential scan: per step one fused multiply+reduce
    (d = k.u) and one fused multiply+add (u += d*ktilde_neg); y is
    accumulated once per chunk from the stored d's.
"""

import numpy as np

B, L, H, V = 1024, 2048, 32, 64
N_CORES = 8
BL = B // N_CORES          # 128 batch lanes per core
T = L - 1                  # 2047 scan steps (keys = positions 0..L-2)
W = 8                      # steps per chunk (one PSUM bank = 8*64 f32)
LN_EPS = 1e-5
DELTA_EPS = 1e-6

_BUILT = {}


def _build_module(t_steps=T, w=W):
    """Build the Bass module (once per process)."""
    import concourse.bass as bass  # noqa: F401
    import concourse.mybir as mybir
    import concourse.tile as tile
    from concourse import bacc
    from concourse.masks import make_identity

    f32 = mybir.dt.float32
    bf16 = mybir.dt.bfloat16
    OP = mybir.AluOpType

    nc = bacc.Bacc("TRN2", target_bir_lowering=False, debug=False,
                   num_devices=N_CORES)

    n_chunks = (t_steps + w - 1) // w
    ncols = n_chunks * w * BL  # one column per (step, batch), padded

    tok = nc.dram_tensor("tok", [V, ncols], bf16, kind="ExternalInput")
    tbl = nc.dram_tensor("tbl", [V, 2 * H], f32, kind="ExternalInput")
    iot = nc.dram_tensor("iot", [V, 1], f32, kind="ExternalInput")  # -v
    qin = nc.dram_tensor("qin", [BL, H], f32, kind="ExternalInput")
    rw = nc.dram_tensor("rw", [H, H], f32, kind="ExternalInput")
    rb = nc.dram_tensor("rb", [H, 1], f32, kind="ExternalInput")
    ow = nc.dram_tensor("ow", [H, V], f32, kind="ExternalInput")
    ob = nc.dram_tensor("ob", [V, 1], f32, kind="ExternalInput")
    outT = nc.dram_tensor("outT", [V, BL], f32, kind="ExternalOutput")

    cw = w * BL  # token columns per chunk

    with tile.TileContext(nc) as tc:
        with (
            tc.tile_pool(name="persist", bufs=1) as persist,
            tc.tile_pool(name="tokp", bufs=4) as tokp,
            tc.tile_pool(name="ohp", bufs=4) as ohp,
            tc.tile_pool(name="kp", bufs=4) as kp,
            tc.tile_pool(name="dpool", bufs=2) as dpool,
            tc.tile_pool(name="spool", bufs=2) as spool,
            tc.tile_pool(name="ypool", bufs=2) as ypool,
            tc.tile_pool(name="psum", bufs=4, space="PSUM") as psum,
            tc.tile_pool(name="psum_r", bufs=1, space="PSUM") as psum_r,
        ):
            u = persist.tile([BL, H], f32)
            nc.sync.dma_start(u[:], qin.ap())
            y = persist.tile([BL, H], f32)
            nc.vector.memset(y[:], 0.0)
            tbl_sb = persist.tile([V, 2 * H], f32)
            nc.sync.dma_start(tbl_sb[:], tbl.ap())
            iota_sb = persist.tile([V, 1], f32)
            nc.sync.dma_start(iota_sb[:], iot.ap())

            rw_sb = persist.tile([H, H], f32)
            nc.sync.dma_start(rw_sb[:], rw.ap())
            rb_sb = persist.tile([H, 1], f32)
            nc.sync.dma_start(rb_sb[:], rb.ap())
            ow_sb = persist.tile([H, V], f32)
            nc.sync.dma_start(ow_sb[:], ow.ap())
            ob_sb = persist.tile([V, 1], f32)
            nc.sync.dma_start(ob_sb[:], ob.ap())
            ident = persist.tile([BL, BL], f32)
            make_identity(nc, ident[:])

            for c in range(n_chunks):
                wc = min(w, t_steps - c * w)
                # token ids for this chunk, replicated across 64 partitions
                tk = tokp.tile([V, cw], bf16, tag="tk")
                nc.sync.dma_start(tk[:], tok.ap()[:, c * cw:(c + 1) * cw])
                # one-hot selectors (f32 0/1) on the scalar engine:
                # relu(1 - |t - v|) is exact for integer-valued t, v
                oht = ohp.tile([V, cw], f32, tag="oht")
                nc.scalar.activation(
                    out=oht[:], in_=tk[:],
                    func=mybir.ActivationFunctionType.Abs,
                    bias=iota_sb[:, 0:1], scale=1.0)
                oh = ohp.tile([V, cw], f32, tag="oh")
                nc.scalar.activation(
                    out=oh[:], in_=oht[:],
                    func=mybir.ActivationFunctionType.Relu,
                    bias=1.0, scale=-1.0)
                # PE: one matmul per step -> k-slab [128b, ktilde|k] in PSUM
                kps = psum.tile([BL, w, 2 * H], f32, tag="kps")
                for j in range(wc):
                    nc.tensor.matmul(
                        out=kps[:, j, :],
                        lhsT=oh[:, j * BL:(j + 1) * BL],
                        rhs=tbl_sb[:],
                        start=True, stop=True)
                # drain chunk to SBUF (scalar engine)
                kt = kp.tile([BL, w, 2 * H], f32, tag="kt")
                nc.scalar.copy(out=kt[:, :wc, :], in_=kps[:, :wc, :])

                db = dpool.tile([BL, w], f32, tag="db")
                for j in range(wc):
                    sc = spool.tile([BL, H], f32, tag="sc")
                    # d_j = sum_h k*u   (out tile is scratch)
                    nc.vector.scalar_tensor_tensor(
                        out=sc[:], in0=kt[:, j, H:2 * H], scalar=1.0,
                        in1=u[:], op0=OP.mult, op1=OP.mult,
                        accum_out=db[:, j:j + 1],
                    )
                    # u += d_j * ktilde_neg_j
                    nc.vector.scalar_tensor_tensor(
                        out=u[:], in0=kt[:, j, 0:H], scalar=db[:, j:j + 1],
                        in1=u[:], op0=OP.mult, op1=OP.add,
                    )
                # y accumulation per chunk: y += sum_j d_j * k_j
                yt = ypool.tile([BL, H, w], f32, tag="yt")
                d_b = db[:, 0:wc].rearrange(
                    "p (s o) -> p o s", o=1).to_broadcast([BL, H, wc])
                k_b = kt[:, 0:wc, H:2 * H].rearrange("p s h -> p h s")
                nc.vector.tensor_tensor(
                    out=yt[:, :, :wc], in0=d_b, in1=k_b, op=OP.mult)
                yp = spool.tile([BL, H], f32, tag="yp")
                nc.vector.tensor_reduce(
                    out=yp[:], in_=yt[:, :, :wc],
                    axis=mybir.AxisListType.X, op=OP.add)
                nc.vector.tensor_tensor(
                    out=y[:], in0=y[:], in1=yp[:], op=OP.add)

            # ---- readout: out = (y @ rw + rb) @ ow + ob, emitted transposed
            yT_ps = psum_r.tile([H, BL], f32, tag="yT")
            nc.tensor.transpose(out=yT_ps[:], in_=y[:], identity=ident[:])
            yT = spool.tile([H, BL], f32, tag="yT_sb")
            nc.scalar.copy(out=yT[:], in_=yT_ps[:])

            r1_ps = psum_r.tile([H, BL], f32, tag="r1")
            nc.tensor.matmul(out=r1_ps[:], lhsT=rw_sb[:], rhs=yT[:],
                             start=True, stop=True)
            r1 = spool.tile([H, BL], f32, tag="r1_sb")
            nc.scalar.add(out=r1[:], in_=r1_ps[:], add=rb_sb[:])

            o_ps = psum_r.tile([V, BL], f32, tag="o")
            nc.tensor.matmul(out=o_ps[:], lhsT=ow_sb[:], rhs=r1[:],
                             start=True, stop=True)
            o_sb = spool.tile([V, BL], f32, tag="o_sb")
            nc.scalar.add(out=o_sb[:], in_=o_ps[:], add=ob_sb[:])
            nc.sync.dma_start(outT.ap(), o_sb[:])

    nc.compile()
    return nc


def _host_tables(embed, w1, b1, w2, b2, ln_g, ln_b):
    """64x32 encoder LUT + the [ -a*k | k ] table, all f32."""
    f = np.float32
    h = embed.astype(f)                      # [64, 32] (ids 0..63)
    ff = np.maximum(h @ w1.astype(f) + b1.astype(f), f(0)) @ w2.astype(f) \
        + b2.astype(f)
    x = h + ff
    mu = x.mean(-1, keepdims=True, dtype=f)
    var = ((x - mu) ** 2).mean(-1, keepdims=True, dtype=f)
    lut = ((x - mu) / np.sqrt(var + f(LN_EPS)) * ln_g.astype(f)
           + ln_b.astype(f)).astype(f)       # [64, 32]
    alpha = f(1.0) / ((lut * lut).sum(-1) + f(DELTA_EPS))   # [64]
    tbl = np.concatenate([-alpha[:, None] * lut, lut], axis=1).astype(f)
    return lut, tbl


def kernel(seq, embed, w1, b1, w2, b2, ln_g, ln_b, read_w, read_b,
           out_w, out_b):
    import ml_dtypes
    from concourse.bass_utils import run_bass_kernel_spmd

    seq = np.asarray(seq)
    lut, tbl = _host_tables(np.asarray(embed), np.asarray(w1), np.asarray(b1),
                            np.asarray(w2), np.asarray(b2),
                            np.asarray(ln_g), np.asarray(ln_b))

    # reversed key order: column g holds the token at position L-2-g
    keys_rev = seq[:, L - 2::-1].astype(np.int32)        # [B, T]
    q_all = lut[seq[:, L - 1]]                           # [B, H] f32

    n_chunks = (T + W - 1) // W
    L2 = n_chunks * W

    rw_np = np.asarray(read_w, np.float32)
    rb_np = np.asarray(read_b, np.float32).reshape(H, 1)
    ow_np = np.asarray(out_w, np.float32)
    ob_np = np.asarray(out_b, np.float32).reshape(V, 1)
    iota = -np.arange(V, dtype=np.float32).reshape(V, 1)

    if "nc" not in _BUILT:
        _BUILT["nc"] = _build_module()
    nc = _BUILT["nc"]

    in_maps = []
    for c in range(N_CORES):
        sl = slice(c * BL, (c + 1) * BL)
        kr = np.full((BL, L2), -1, np.int32)
        kr[:, :T] = keys_rev[sl]
        # column order: g-major, batch-minor; replicated over 64 partitions
        cols = kr.T.ravel().astype(np.float32).astype(ml_dtypes.bfloat16)
        tok = np.broadcast_to(cols[None, :], (V, L2 * BL))
        in_maps.append({
            "tok": np.ascontiguousarray(tok),
            "tbl": tbl,
            "iot": iota,
            "qin": np.ascontiguousarray(q_all[sl]),
            "rw": rw_np, "rb": rb_np, "ow": ow_np, "ob": ob_np,
        })

    import os
    trace = os.environ.get("KERNEL_TRACE", "0") == "1"
    res = run_bass_kernel_spmd(nc, in_maps, core_ids=list(range(N_CORES)),
                               trace=trace)
    _BUILT["last_result"] = res
    out = np.empty((B, V), np.float32)
    for c in range(N_CORES):
        out[c * BL:(c + 1) * BL] = res.results[c]["outT"].T
    return out


# revision 36
# speedup vs baseline: 3.4653x; 1.1466x over previous
"""Trainium2 Bass kernel for nn_DeltaRuleModel (scatter_memory).

Model: token embed -> per-token MLP+LayerNorm encoder -> sequential
delta-rule memory scan over L-1 steps -> readout of the final memory
against the last position's hidden -> 2 small dense layers.

Key algebraic facts exploited:
  1. The encoder output hidden[b, l] depends only on the token id
     seq[b, l]  =>  the whole encoder collapses to a 64x32 table (TBL),
     computed on the host from the small weights (pure weight
     preprocessing; all per-token work stays on device).
  2. The scan M <- M (I - a k k^T) + k k^T with the final readout
     y = M_T q is linear in M, so y equals a backward *vector*
     recurrence (no 32x32 matrix state):
         u <- q;  for s = T..1:  d = k_s.u ; y += d k_s ; u -= a_s d k_s
     This is 2 fused DVE ops per step on [128, 32] tiles (batch on
     partitions) instead of a 32x32 matrix update.

Per-core dataflow (128 batch lanes on partitions):
  - GPSIMD builds one-hot selectors from replicated token ids
    (is_equal against a per-partition iota), f32.
  - PE turns each step's one-hot [64v x 128b] into the k-slab
    [128b x (ktilde|k)] via a matmul against the 64x64 table ->
    an on-chip "gather" at matmul speed, no DMA descriptors.
  - ACT drains PSUM k-slabs to SBUF once per chunk.
  - DVE runs the sequential scan: per step one fused multiply+reduce
    (d = k.u) and one fused multiply+add (u += d*ktilde_neg); y is
    accumulated once per chunk from the stored d's.
"""

import numpy as np

B, L, H, V = 1024, 2048, 32, 64
N_CORES = 8
BL = B // N_CORES          # 128 batch lanes per core
T = L - 1                  # 2047 scan steps (keys = positions 0..L-2)
W = 8                      # steps per chunk (one PSUM bank = 8*64 f32)
LN_EPS = 1e-5
DELTA_EPS = 1e-6

_BUILT = {}


def _build_module(t_steps=T, w=W):
    """Build the Bass module (once per process)."""
    import concourse.bass as bass  # noqa: F401
    import concourse.mybir as mybir
    import concourse.tile as tile
    from concourse import bacc
    from concourse.masks import make_identity

    f32 = mybir.dt.float32
    bf16 = mybir.dt.bfloat16
    OP = mybir.AluOpType

    nc = bacc.Bacc("TRN2", target_bir_lowering=False, debug=False,
                   num_devices=N_CORES)

    # steps are processed in PAIRS: one PE matmul materializes two steps'
    # k-vectors using the full 128-partition contraction (stacked one-hots
    # against a block-diagonal [TBL 0; 0 TBL] moving tensor).
    n_pairs = (t_steps + 1) // 2
    n_chunks = (n_pairs + w - 1) // w          # w PAIRS per chunk
    ncols = n_chunks * w * BL                  # one column per (pair, batch)

    tok = nc.dram_tensor("tok", [2 * V, ncols], bf16, kind="ExternalInput")
    tbl = nc.dram_tensor("tbl", [2 * V, 4 * H], f32, kind="ExternalInput")
    iot = nc.dram_tensor("iot", [2 * V, 1], f32, kind="ExternalInput")  # -v
    qin = nc.dram_tensor("qin", [BL, H], f32, kind="ExternalInput")
    rw = nc.dram_tensor("rw", [H, H], f32, kind="ExternalInput")
    rb = nc.dram_tensor("rb", [H, 1], f32, kind="ExternalInput")
    ow = nc.dram_tensor("ow", [H, V], f32, kind="ExternalInput")
    ob = nc.dram_tensor("ob", [V, 1], f32, kind="ExternalInput")
    outT = nc.dram_tensor("outT", [V, BL], f32, kind="ExternalOutput")

    cw = w * BL  # token-pair columns per chunk

    with tile.TileContext(nc) as tc:
        with (
            tc.tile_pool(name="persist", bufs=1) as persist,
            tc.tile_pool(name="tokp", bufs=4) as tokp,
            tc.tile_pool(name="ohp", bufs=4) as ohp,
            tc.tile_pool(name="kp", bufs=4) as kp,
            tc.tile_pool(name="dpool", bufs=2) as dpool,
            tc.tile_pool(name="spool", bufs=2) as spool,
            tc.tile_pool(name="ypool", bufs=2) as ypool,
            tc.tile_pool(name="psum", bufs=2, space="PSUM") as psum,
            tc.tile_pool(name="psum_r", bufs=1, space="PSUM") as psum_r,
        ):
            u = persist.tile([BL, H], f32)
            nc.sync.dma_start(u[:], qin.ap())
            y = persist.tile([BL, H], f32)
            nc.vector.memset(y[:], 0.0)
            tbl_sb = persist.tile([2 * V, 4 * H], f32)
            nc.sync.dma_start(tbl_sb[:], tbl.ap())
            iota_sb = persist.tile([2 * V, 1], f32)
            nc.sync.dma_start(iota_sb[:], iot.ap())

            rw_sb = persist.tile([H, H], f32)
            nc.sync.dma_start(rw_sb[:], rw.ap())
            rb_sb = persist.tile([H, 1], f32)
            nc.sync.dma_start(rb_sb[:], rb.ap())
            ow_sb = persist.tile([H, V], f32)
            nc.sync.dma_start(ow_sb[:], ow.ap())
            ob_sb = persist.tile([V, 1], f32)
            nc.sync.dma_start(ob_sb[:], ob.ap())
            ident = persist.tile([BL, BL], f32)
            make_identity(nc, ident[:])

            # y partials, kept unreduced [b, h, step-in-chunk]; reduced once
            ybig = persist.tile([BL, H, 2 * w], f32)
            nc.gpsimd.memset(ybig[:], 0.0)

            for c in range(n_chunks):
                pc = min(w, n_pairs - c * w)         # pairs this chunk
                nst = min(2 * w, t_steps - c * 2 * w)  # steps this chunk
                # stacked token-pair ids (even step in rows 0:64, odd in
                # 64:128), one column per (pair, batch)
                tk = tokp.tile([2 * V, cw], bf16, tag="tk")
                nc.sync.dma_start(tk[:], tok.ap()[:, c * cw:(c + 1) * cw])
                # one-hot selectors (f32 0/1) on the scalar engine:
                # relu(1 - |t - v|) is exact for integer-valued t, v
                oht = ohp.tile([2 * V, cw], f32, tag="oht")
                nc.scalar.activation(
                    out=oht[:], in_=tk[:],
                    func=mybir.ActivationFunctionType.Abs,
                    bias=iota_sb[:, 0:1], scale=1.0)
                oh = ohp.tile([2 * V, cw], f32, tag="oh")
                nc.scalar.activation(
                    out=oh[:], in_=oht[:],
                    func=mybir.ActivationFunctionType.Relu,
                    bias=1.0, scale=-1.0)
                # PE: one matmul per PAIR -> [128b, ktilde_e|k_e|ktilde_o|k_o]
                kps = psum.tile([BL, w, 4 * H], f32, tag="kps")
                for j in range(pc):
                    nc.tensor.matmul(
                        out=kps[:, j, :],
                        lhsT=oh[:, j * BL:(j + 1) * BL],
                        rhs=tbl_sb[:],
                        start=True, stop=True)
                # drain chunk to SBUF (scalar engine)
                kt = kp.tile([BL, w, 4 * H], f32, tag="kt")
                nc.scalar.copy(out=kt[:, :pc, :], in_=kps[:, :pc, :])

                db = dpool.tile([BL, 2 * w], f32, tag="db")
                for s in range(nst):
                    j, odd = divmod(s, 2)
                    o = 2 * H * odd
                    sc = spool.tile([BL, H], f32, tag="sc")
                    # d_s = sum_h k*u   (out tile is scratch)
                    nc.vector.scalar_tensor_tensor(
                        out=sc[:], in0=kt[:, j, o + H:o + 2 * H], scalar=1.0,
                        in1=u[:], op0=OP.mult, op1=OP.mult,
                        accum_out=db[:, s:s + 1],
                    )
                    # u += d_s * ktilde_neg_s
                    nc.vector.scalar_tensor_tensor(
                        out=u[:], in0=kt[:, j, o:o + H], scalar=db[:, s:s + 1],
                        in1=u[:], op0=OP.mult, op1=OP.add,
                    )
                # y partials per chunk on GPSIMD: ybig[:, :, s] += d_s * k_s
                # view kt as [BL, 2w, 64] so k_s = kv[:, s, 32:64]
                kv = kt[:].rearrange("p a (t b) -> p (a t) b", t=2)
                yt = ypool.tile([BL, H, 2 * w], f32, tag="yt")
                d_b = db[:, 0:nst].rearrange(
                    "p (s o) -> p o s", o=1).to_broadcast([BL, H, nst])
                k_b = kv[:, 0:nst, H:2 * H].rearrange("p s h -> p h s")
                nc.gpsimd.tensor_tensor(
                    out=yt[:, :, :nst], in0=d_b, in1=k_b, op=OP.mult)
                nc.gpsimd.tensor_tensor(
                    out=ybig[:, :, :nst], in0=ybig[:, :, :nst],
                    in1=yt[:, :, :nst], op=OP.add)
            nc.vector.tensor_reduce(
                out=y[:], in_=ybig[:],
                axis=mybir.AxisListType.X, op=OP.add)

            # ---- readout: out = (y @ rw + rb) @ ow + ob, emitted transposed
            yT_ps = psum_r.tile([H, BL], f32, tag="yT")
            nc.tensor.transpose(out=yT_ps[:], in_=y[:], identity=ident[:])
            yT = spool.tile([H, BL], f32, tag="yT_sb")
            nc.scalar.copy(out=yT[:], in_=yT_ps[:])

            r1_ps = psum_r.tile([H, BL], f32, tag="r1")
            nc.tensor.matmul(out=r1_ps[:], lhsT=rw_sb[:], rhs=yT[:],
                             start=True, stop=True)
            r1 = spool.tile([H, BL], f32, tag="r1_sb")
            nc.scalar.add(out=r1[:], in_=r1_ps[:], add=rb_sb[:])

            o_ps = psum_r.tile([V, BL], f32, tag="o")
            nc.tensor.matmul(out=o_ps[:], lhsT=ow_sb[:], rhs=r1[:],
                             start=True, stop=True)
            o_sb = spool.tile([V, BL], f32, tag="o_sb")
            nc.scalar.add(out=o_sb[:], in_=o_ps[:], add=ob_sb[:])
            nc.sync.dma_start(outT.ap(), o_sb[:])

    nc.compile()
    return nc


def _host_tables(embed, w1, b1, w2, b2, ln_g, ln_b):
    """64x32 encoder LUT + the [ -a*k | k ] table, all f32."""
    f = np.float32
    h = embed.astype(f)                      # [64, 32] (ids 0..63)
    ff = np.maximum(h @ w1.astype(f) + b1.astype(f), f(0)) @ w2.astype(f) \
        + b2.astype(f)
    x = h + ff
    mu = x.mean(-1, keepdims=True, dtype=f)
    var = ((x - mu) ** 2).mean(-1, keepdims=True, dtype=f)
    lut = ((x - mu) / np.sqrt(var + f(LN_EPS)) * ln_g.astype(f)
           + ln_b.astype(f)).astype(f)       # [64, 32]
    alpha = f(1.0) / ((lut * lut).sum(-1) + f(DELTA_EPS))   # [64]
    tbl = np.concatenate([-alpha[:, None] * lut, lut], axis=1).astype(f)
    return lut, tbl


def kernel(seq, embed, w1, b1, w2, b2, ln_g, ln_b, read_w, read_b,
           out_w, out_b):
    import ml_dtypes
    from concourse.bass_utils import run_bass_kernel_spmd

    seq = np.asarray(seq)
    lut, tbl = _host_tables(np.asarray(embed), np.asarray(w1), np.asarray(b1),
                            np.asarray(w2), np.asarray(b2),
                            np.asarray(ln_g), np.asarray(ln_b))

    # reversed key order: column g holds the token at position L-2-g
    keys_rev = seq[:, L - 2::-1].astype(np.int32)        # [B, T]
    q_all = lut[seq[:, L - 1]]                           # [B, H] f32

    n_pairs = (T + 1) // 2
    n_chunks = (n_pairs + W - 1) // W
    P2 = n_chunks * W                                    # padded pairs

    rw_np = np.asarray(read_w, np.float32)
    rb_np = np.asarray(read_b, np.float32).reshape(H, 1)
    ow_np = np.asarray(out_w, np.float32)
    ob_np = np.asarray(out_b, np.float32).reshape(V, 1)
    iota = -np.concatenate([np.arange(V), np.arange(V)]) \
        .astype(np.float32).reshape(2 * V, 1)
    # block-diagonal moving tensor [TBL 0; 0 TBL]
    tbl2 = np.zeros((2 * V, 4 * H), np.float32)
    tbl2[:V, :2 * H] = tbl
    tbl2[V:, 2 * H:] = tbl

    if "nc" not in _BUILT:
        _BUILT["nc"] = _build_module()
    nc = _BUILT["nc"]

    in_maps = []
    for c in range(N_CORES):
        sl = slice(c * BL, (c + 1) * BL)
        kr = np.full((BL, 2 * P2), -1, np.int32)
        kr[:, :T] = keys_rev[sl]
        ev = kr[:, 0::2]                   # [BL, P2] even-step tokens
        od = kr[:, 1::2]                   # [BL, P2] odd-step tokens
        # column order: pair-major, batch-minor
        evc = ev.T.ravel().astype(np.float32).astype(ml_dtypes.bfloat16)
        odc = od.T.ravel().astype(np.float32).astype(ml_dtypes.bfloat16)
        tok = np.empty((2 * V, P2 * BL), ml_dtypes.bfloat16)
        tok[:V] = np.broadcast_to(evc[None, :], (V, P2 * BL))
        tok[V:] = np.broadcast_to(odc[None, :], (V, P2 * BL))
        in_maps.append({
            "tok": np.ascontiguousarray(tok),
            "tbl": tbl2,
            "iot": iota,
            "qin": np.ascontiguousarray(q_all[sl]),
            "rw": rw_np, "rb": rb_np, "ow": ow_np, "ob": ob_np,
        })

    import os
    trace = os.environ.get("KERNEL_TRACE", "0") == "1"
    res = run_bass_kernel_spmd(nc, in_maps, core_ids=list(range(N_CORES)),
                               trace=trace)
    _BUILT["last_result"] = res
    out = np.empty((B, V), np.float32)
    for c in range(N_CORES):
        out[c * BL:(c + 1) * BL] = res.results[c]["outT"].T
    return out


# revision 38
# speedup vs baseline: 4.3058x; 1.2425x over previous
"""Trainium2 Bass kernel for nn_DeltaRuleModel (scatter_memory).

Model: token embed -> per-token MLP+LayerNorm encoder -> sequential
delta-rule memory scan over L-1 steps -> readout of the final memory
against the last position's hidden -> 2 small dense layers.

Key algebraic facts exploited:
  1. The encoder output hidden[b, l] depends only on the token id
     seq[b, l]  =>  the whole encoder collapses to a 64x32 table (TBL),
     computed on the host from the small weights (pure weight
     preprocessing; all per-token work stays on device).
  2. The scan M <- M (I - a k k^T) + k k^T with the final readout
     y = M_T q is linear in M, so y equals a backward *vector*
     recurrence (no 32x32 matrix state):
         u <- q;  for s = T..1:  d = k_s.u ; y += d k_s ; u -= a_s d k_s
     This is 2 fused DVE ops per step on [128, 32] tiles (batch on
     partitions) instead of a 32x32 matrix update.

Per-core dataflow (128 batch lanes on partitions):
  - ACT builds one-hot selectors from replicated token ids in two exact
    passes: |t - v| then relu(1 - x)  (f32 0/1).
  - PE materializes TWO steps' k-vectors per matmul ("pair stacking"):
    lhsT = stacked one-hots [128(2v) x 128b], moving = block-diag
    [TBL 0; 0 TBL] -> [128b x (ktilde_e|k_e|ktilde_o|k_o)] in PSUM.
    This is an on-chip table gather at matmul speed, no DMA descriptors.
  - ACT drains PSUM k-slabs to SBUF once per chunk.
  - DVE runs the sequential scan: per step one fused multiply+reduce
    (d = k.u, via scalar_tensor_tensor accum_out) and one fused
    multiply+add (u += d*ktilde_neg).
  - GPSIMD accumulates the y partials (d_s * k_s) per chunk; one final
    DVE reduce produces y, then a small PE readout emits out^T.
"""

import numpy as np

B, L, H, V = 1024, 2048, 32, 64
N_CORES = 8
BL = B // N_CORES          # 128 batch lanes per core
T = L - 1                  # 2047 scan steps (keys = positions 0..L-2)
W = 8                      # steps per chunk (one PSUM bank = 8*64 f32)
LN_EPS = 1e-5
DELTA_EPS = 1e-6

_BUILT = {}


def _build_module(t_steps=T, w=W):
    """Build the Bass module (once per process)."""
    import concourse.bass as bass  # noqa: F401
    import concourse.mybir as mybir
    import concourse.tile as tile
    from concourse import bacc
    from concourse.masks import make_identity

    f32 = mybir.dt.float32
    bf16 = mybir.dt.bfloat16
    OP = mybir.AluOpType

    nc = bacc.Bacc("TRN2", target_bir_lowering=False, debug=False,
                   num_devices=N_CORES)

    # steps are processed in PAIRS: one PE matmul materializes two steps'
    # k-vectors using the full 128-partition contraction (stacked one-hots
    # against a block-diagonal [TBL 0; 0 TBL] moving tensor).
    n_pairs = (t_steps + 1) // 2
    n_chunks = (n_pairs + w - 1) // w          # w PAIRS per chunk
    ncols = n_chunks * w * BL                  # one column per (pair, batch)

    tok = nc.dram_tensor("tok", [2 * V, ncols], bf16, kind="ExternalInput")
    tbl = nc.dram_tensor("tbl", [2 * V, 4 * H], f32, kind="ExternalInput")
    iot = nc.dram_tensor("iot", [2 * V, 1], f32, kind="ExternalInput")  # -v
    qin = nc.dram_tensor("qin", [BL, H], f32, kind="ExternalInput")
    rw = nc.dram_tensor("rw", [H, H], f32, kind="ExternalInput")
    rb = nc.dram_tensor("rb", [H, 1], f32, kind="ExternalInput")
    ow = nc.dram_tensor("ow", [H, V], f32, kind="ExternalInput")
    ob = nc.dram_tensor("ob", [V, 1], f32, kind="ExternalInput")
    outT = nc.dram_tensor("outT", [V, BL], f32, kind="ExternalOutput")

    cw = w * BL  # token-pair columns per chunk

    with tile.TileContext(nc) as tc:
        with (
            tc.tile_pool(name="persist", bufs=1) as persist,
            tc.tile_pool(name="tokp", bufs=4) as tokp,
            tc.tile_pool(name="ohp", bufs=4) as ohp,
            tc.tile_pool(name="kp", bufs=4) as kp,
            tc.tile_pool(name="dpool", bufs=2) as dpool,
            tc.tile_pool(name="spool", bufs=2) as spool,
            tc.tile_pool(name="ypool", bufs=2) as ypool,
            tc.tile_pool(name="psum", bufs=2, space="PSUM") as psum,
            tc.tile_pool(name="psum_r", bufs=1, space="PSUM") as psum_r,
        ):
            u = persist.tile([BL, H], f32)
            nc.sync.dma_start(u[:], qin.ap())
            y = persist.tile([BL, H], f32)
            nc.vector.memset(y[:], 0.0)
            tbl_sb = persist.tile([2 * V, 4 * H], f32)
            nc.sync.dma_start(tbl_sb[:], tbl.ap())
            iota_sb = persist.tile([2 * V, 1], f32)
            nc.sync.dma_start(iota_sb[:], iot.ap())

            rw_sb = persist.tile([H, H], f32)
            nc.sync.dma_start(rw_sb[:], rw.ap())
            rb_sb = persist.tile([H, 1], f32)
            nc.sync.dma_start(rb_sb[:], rb.ap())
            ow_sb = persist.tile([H, V], f32)
            nc.sync.dma_start(ow_sb[:], ow.ap())
            ob_sb = persist.tile([V, 1], f32)
            nc.sync.dma_start(ob_sb[:], ob.ap())
            ident = persist.tile([BL, BL], f32)
            make_identity(nc, ident[:])

            # y partials, kept unreduced [b, h, step-in-chunk]; reduced once
            ybig = persist.tile([BL, H, 2 * w], f32)
            nc.gpsimd.memset(ybig[:], 0.0)

            for c in range(n_chunks):
                pc = min(w, n_pairs - c * w)         # pairs this chunk
                nst = min(2 * w, t_steps - c * 2 * w)  # steps this chunk
                # stacked token-pair ids (even step in rows 0:64, odd in
                # 64:128), one column per (pair, batch)
                tk = tokp.tile([2 * V, cw], bf16, tag="tk")
                nc.sync.dma_start(tk[:], tok.ap()[:, c * cw:(c + 1) * cw])
                # one-hot selectors (f32 0/1) on the scalar engine:
                # relu(1 - |t - v|) is exact for integer-valued t, v
                oht = ohp.tile([2 * V, cw], f32, tag="oht")
                nc.scalar.activation(
                    out=oht[:], in_=tk[:],
                    func=mybir.ActivationFunctionType.Abs,
                    bias=iota_sb[:, 0:1], scale=1.0)
                oh = ohp.tile([2 * V, cw], f32, tag="oh")
                nc.scalar.activation(
                    out=oh[:], in_=oht[:],
                    func=mybir.ActivationFunctionType.Relu,
                    bias=1.0, scale=-1.0)
                # PE: one matmul per PAIR -> [128b, ktilde_e|k_e|ktilde_o|k_o]
                kps = psum.tile([BL, w, 4 * H], f32, tag="kps")
                for j in range(pc):
                    nc.tensor.matmul(
                        out=kps[:, j, :],
                        lhsT=oh[:, j * BL:(j + 1) * BL],
                        rhs=tbl_sb[:],
                        start=True, stop=True)
                # drain chunk to SBUF (scalar engine)
                kt = kp.tile([BL, w, 4 * H], f32, tag="kt")
                nc.scalar.copy(out=kt[:, :pc, :], in_=kps[:, :pc, :])

                db = dpool.tile([BL, 2 * w], f32, tag="db")
                for s in range(nst):
                    j, odd = divmod(s, 2)
                    o = 2 * H * odd
                    sc = spool.tile([BL, H], f32, tag="sc")
                    # d_s = sum_h k*u (read k straight from PSUM; the SBUF
                    # drain only feeds the y-ops, off this critical chain)
                    nc.vector.scalar_tensor_tensor(
                        out=sc[:], in0=kps[:, j, o + H:o + 2 * H], scalar=1.0,
                        in1=u[:], op0=OP.mult, op1=OP.mult,
                        accum_out=db[:, s:s + 1],
                    )
                    # u += d_s * ktilde_neg_s
                    nc.vector.scalar_tensor_tensor(
                        out=u[:], in0=kps[:, j, o:o + H], scalar=db[:, s:s + 1],
                        in1=u[:], op0=OP.mult, op1=OP.add,
                    )
                # y partials per chunk on GPSIMD: ybig[:, :, s] += d_s * k_s
                # view kt as [BL, 2w, 64] so k_s = kv[:, s, 32:64]
                kv = kt[:].rearrange("p a (t b) -> p (a t) b", t=2)
                yt = ypool.tile([BL, H, 2 * w], f32, tag="yt")
                d_b = db[:, 0:nst].rearrange(
                    "p (s o) -> p o s", o=1).to_broadcast([BL, H, nst])
                k_b = kv[:, 0:nst, H:2 * H].rearrange("p s h -> p h s")
                nc.gpsimd.tensor_tensor(
                    out=yt[:, :, :nst], in0=d_b, in1=k_b, op=OP.mult)
                nc.gpsimd.tensor_tensor(
                    out=ybig[:, :, :nst], in0=ybig[:, :, :nst],
                    in1=yt[:, :, :nst], op=OP.add)
            nc.vector.tensor_reduce(
                out=y[:], in_=ybig[:],
                axis=mybir.AxisListType.X, op=OP.add)

            # ---- readout: out = (y @ rw + rb) @ ow + ob, emitted transposed
            yT_ps = psum_r.tile([H, BL], f32, tag="yT")
            nc.tensor.transpose(out=yT_ps[:], in_=y[:], identity=ident[:])
            yT = spool.tile([H, BL], f32, tag="yT_sb")
            nc.scalar.copy(out=yT[:], in_=yT_ps[:])

            r1_ps = psum_r.tile([H, BL], f32, tag="r1")
            nc.tensor.matmul(out=r1_ps[:], lhsT=rw_sb[:], rhs=yT[:],
                             start=True, stop=True)
            r1 = spool.tile([H, BL], f32, tag="r1_sb")
            nc.scalar.add(out=r1[:], in_=r1_ps[:], add=rb_sb[:])

            o_ps = psum_r.tile([V, BL], f32, tag="o")
            nc.tensor.matmul(out=o_ps[:], lhsT=ow_sb[:], rhs=r1[:],
                             start=True, stop=True)
            o_sb = spool.tile([V, BL], f32, tag="o_sb")
            nc.scalar.add(out=o_sb[:], in_=o_ps[:], add=ob_sb[:])
            nc.sync.dma_start(outT.ap(), o_sb[:])

    nc.compile()
    return nc


def _host_tables(embed, w1, b1, w2, b2, ln_g, ln_b):
    """64x32 encoder LUT + the [ -a*k | k ] table, all f32."""
    f = np.float32
    h = embed.astype(f)                      # [64, 32] (ids 0..63)
    ff = np.maximum(h @ w1.astype(f) + b1.astype(f), f(0)) @ w2.astype(f) \
        + b2.astype(f)
    x = h + ff
    mu = x.mean(-1, keepdims=True, dtype=f)
    var = ((x - mu) ** 2).mean(-1, keepdims=True, dtype=f)
    lut = ((x - mu) / np.sqrt(var + f(LN_EPS)) * ln_g.astype(f)
           + ln_b.astype(f)).astype(f)       # [64, 32]
    alpha = f(1.0) / ((lut * lut).sum(-1) + f(DELTA_EPS))   # [64]
    tbl = np.concatenate([-alpha[:, None] * lut, lut], axis=1).astype(f)
    return lut, tbl


def kernel(seq, embed, w1, b1, w2, b2, ln_g, ln_b, read_w, read_b,
           out_w, out_b):
    import ml_dtypes
    from concourse.bass_utils import run_bass_kernel_spmd

    seq = np.asarray(seq)
    lut, tbl = _host_tables(np.asarray(embed), np.asarray(w1), np.asarray(b1),
                            np.asarray(w2), np.asarray(b2),
                            np.asarray(ln_g), np.asarray(ln_b))

    # reversed key order: column g holds the token at position L-2-g
    keys_rev = seq[:, L - 2::-1].astype(np.int32)        # [B, T]
    q_all = lut[seq[:, L - 1]]                           # [B, H] f32

    n_pairs = (T + 1) // 2
    n_chunks = (n_pairs + W - 1) // W
    P2 = n_chunks * W                                    # padded pairs

    rw_np = np.asarray(read_w, np.float32)
    rb_np = np.asarray(read_b, np.float32).reshape(H, 1)
    ow_np = np.asarray(out_w, np.float32)
    ob_np = np.asarray(out_b, np.float32).reshape(V, 1)
    iota = -np.concatenate([np.arange(V), np.arange(V)]) \
        .astype(np.float32).reshape(2 * V, 1)
    # block-diagonal moving tensor [TBL 0; 0 TBL]
    tbl2 = np.zeros((2 * V, 4 * H), np.float32)
    tbl2[:V, :2 * H] = tbl
    tbl2[V:, 2 * H:] = tbl

    if "nc" not in _BUILT:
        _BUILT["nc"] = _build_module()
    nc = _BUILT["nc"]

    in_maps = []
    for c in range(N_CORES):
        sl = slice(c * BL, (c + 1) * BL)
        kr = np.full((BL, 2 * P2), -1, np.int32)
        kr[:, :T] = keys_rev[sl]
        ev = kr[:, 0::2]                   # [BL, P2] even-step tokens
        od = kr[:, 1::2]                   # [BL, P2] odd-step tokens
        # column order: pair-major, batch-minor
        evc = ev.T.ravel().astype(np.float32).astype(ml_dtypes.bfloat16)
        odc = od.T.ravel().astype(np.float32).astype(ml_dtypes.bfloat16)
        tok = np.empty((2 * V, P2 * BL), ml_dtypes.bfloat16)
        tok[:V] = np.broadcast_to(evc[None, :], (V, P2 * BL))
        tok[V:] = np.broadcast_to(odc[None, :], (V, P2 * BL))
        in_maps.append({
            "tok": np.ascontiguousarray(tok),
            "tbl": tbl2,
            "iot": iota,
            "qin": np.ascontiguousarray(q_all[sl]),
            "rw": rw_np, "rb": rb_np, "ow": ow_np, "ob": ob_np,
        })

    import os
    trace = os.environ.get("KERNEL_TRACE", "0") == "1"
    res = run_bass_kernel_spmd(nc, in_maps, core_ids=list(range(N_CORES)),
                               trace=trace)
    _BUILT["last_result"] = res
    out = np.empty((B, V), np.float32)
    for c in range(N_CORES):
        out[c * BL:(c + 1) * BL] = res.results[c]["outT"].T
    return out
